# revision 2
# baseline (speedup 1.0000x reference)
"""AttentionSink masked-add kernel for 8 TRN2 NeuronCores.

out[b,h,i,j] = w[b,h,i,j] + mask[i,j], mask 0 where allowed else -1e30.
Allowed: j < 4 (sink) or i-25 <= j <= i (local band).

Since |w| << ulp(-1e30) in fp32, masked outputs are exactly -1e30, so the
kernel only reads the sink columns and the diagonal band (~1.5% of input)
and writes the full output from SBUF tiles whose background is -1e30.
"""

import numpy as np

import concourse.bass as bass
import concourse.tile as tile
from concourse import bacc, mybir
from concourse.bass_utils import run_bass_kernel_spmd

B, H, S = 4, 16, 2048
SINK = 4
LEFT = 25
NEG = -1e30
P = 128                    # SBUF partitions / rows per block
NBLK = S // P              # 16 row blocks per matrix
N_CORES = 8
M = (B * H) // N_CORES     # matrices per core
BANDW = P + LEFT           # 153: band slab width for blocks r >= 1
NBUF = 2                   # row tiles per matrix slot (ping-pong)


def _host_masks():
    # mask_first: rows 0..127 x cols 0..127 (sink + clamped band; block 0)
    i = np.arange(P)[:, None]
    j = np.arange(P)[None, :]
    allowed0 = (j < SINK) | ((j >= i - LEFT) & (j <= i))
    mask_first = np.where(allowed0, 0.0, NEG).astype(np.float32)
    # mask_band: for blocks r>=1, slab col q maps to j = r*128-25+q, row p to
    # i = r*128+p; allowed iff p <= q <= p+25.
    q = np.arange(BANDW)[None, :]
    allowed = (q >= i) & (q <= i + LEFT)
    mask_band = np.where(allowed, 0.0, NEG).astype(np.float32)
    return mask_first, mask_band


def _build_program():
    nc = bacc.Bacc(
        "TRN2", target_bir_lowering=False, debug=False, num_devices=N_CORES
    )
    dt = mybir.dt.float32
    x = nc.dram_tensor("x", [M, S, S], dt, kind="ExternalInput").ap()
    mf = nc.dram_tensor("mask_first", [P, P], dt, kind="ExternalInput").ap()
    mb = nc.dram_tensor("mask_band", [P, BANDW], dt, kind="ExternalInput").ap()
    out = nc.dram_tensor("out", [M, S, S], dt, kind="ExternalOutput").ap()

    with tile.TileContext(nc) as tc:
        with (
            tc.tile_pool(name="masks", bufs=1) as maskpool,
            tc.tile_pool(name="rows", bufs=1) as rowpool,
            tc.tile_pool(name="band", bufs=4 * M) as bandpool,
        ):
            mf_t = maskpool.tile([P, P], dt, tag="mf")
            nc.sync.dma_start(mf_t[:], mf[:])
            mb_t = maskpool.tile([P, BANDW], dt, tag="mb")
            nc.sync.dma_start(mb_t[:], mb[:])

            # Persistent row tiles: NBUF per matrix, background -1e30.
            T = [
                [
                    rowpool.tile(
                        [P, S], dt, tag=f"T{m}_{k}", name=f"T{m}_{k}"
                    )
                    for k in range(NBUF)
                ]
                for m in range(M)
            ]
            for m in range(M):
                for k in range(NBUF):
                    eng = nc.vector if (m * NBUF + k) % 2 == 0 else nc.gpsimd
                    eng.memset(T[m][k][:], NEG)

            # prev[m][k] = column interval this slot's band last wrote
            prev = [[None] * NBUF for _ in range(M)]

            for r in range(NBLK):
                R = r * P
                k = r % NBUF
                if r == 0:
                    c0, c1, mtile = 0, P, mf_t
                else:
                    c0, c1, mtile = R - LEFT, R + P, mb_t
                for m in range(M):
                    t = T[m][k]
                    # clear this slot's stale band (minus what the new band
                    # and sink DMA overwrite)
                    if prev[m][k] is not None:
                        lo, hi = prev[m][k]
                        lo = max(lo, SINK if r > 0 else 0)
                        hi = min(hi, c0)
                        if lo < hi:
                            nc.vector.memset(t[:, lo:hi], NEG)
                    if r > 0:
                        # sink columns pass through unchanged (mask 0)
                        nc.sync.dma_start(t[:, 0:SINK], x[m, R : R + P, 0:SINK])
                    bt = bandpool.tile([P, c1 - c0], dt, tag="bt")
                    nc.sync.dma_start(bt[:], x[m, R : R + P, c0:c1])
                    nc.vector.tensor_add(t[:, c0:c1], bt[:], mtile[:])
                    nc.sync.dma_start(out[m, R : R + P, :], t[:])
                    prev[m][k] = (c0, c1)

    nc.compile()
    return nc


_CACHE = {}


def _get_nc():
    if "nc" not in _CACHE:
        _CACHE["nc"] = _build_program()
    return _CACHE["nc"]


def kernel(attention_weights, seq_len=None):
    w = np.ascontiguousarray(np.asarray(attention_weights, dtype=np.float32))
    assert w.shape == (B, H, S, S)
    nc = _get_nc()
    mask_first, mask_band = _host_masks()
    flat = w.reshape(B * H, S, S)
    in_maps = [
        {
            "x": flat[i * M : (i + 1) * M],
            "mask_first": mask_first,
            "mask_band": mask_band,
        }
        for i in range(N_CORES)
    ]
    res = run_bass_kernel_spmd(nc, in_maps, core_ids=list(range(N_CORES)))
    out = np.concatenate([res.results[i]["out"] for i in range(N_CORES)], axis=0)
    return out.reshape(B, H, S, S)


# revision 7
# speedup vs baseline: 1.3638x; 1.3638x over previous
"""AttentionSink masked-add kernel for 8 TRN2 NeuronCores.

out[b,h,i,j] = w[b,h,i,j] + mask[i,j], mask 0 where allowed else -1e30.
Allowed: j < 4 (sink) or i-25 <= j <= i (local band).

Since |w| << ulp(-1e30) in fp32, masked outputs are exactly -1e30. The
kernel therefore:
  1. writes the whole output with the constant -1e30 from a small SBUF tile
     (stride-0 broadcast DMA source, full 8 KiB rows, ~peak HBM write BW);
  2. overwrites the allowed positions by copying them straight from the
     input: the sink columns as a thin DRAM->DRAM copy, and the local band
     as a DRAM->DRAM copy over a diagonal access pattern (stride S+1), which
     covers exactly the 26-wide allowed parallelogram for row blocks r>=1 —
     no arithmetic needed since the mask is 0 there.
Only block r=0 (rows 0..127, where the band clips at column 0) goes through
SBUF with a real mask add. Total HBM traffic per core: ~134 MB written +
~11 MB read, ~1.5% of the input read.

The 64 (S,S) matrices are split 8 per core; no collectives.
"""

import numpy as np

import concourse.bass as bass
import concourse.tile as tile
from concourse import bacc, mybir
from concourse.bass_utils import run_bass_kernel_spmd

B, H, S = 4, 16, 2048
SINK = 4
LEFT = 25
NEG = -1e30
P = 128                    # SBUF partitions / rows per block
NBLK = S // P              # 16 row blocks per matrix
N_CORES = 8
M = (B * H) // N_CORES     # matrices per core


def _host_masks():
    # mask for rows 0..127 x cols 0..127 (sink + clamped band; block 0)
    i = np.arange(P)[:, None]
    j = np.arange(P)[None, :]
    allowed0 = (j < SINK) | ((j >= i - LEFT) & (j <= i))
    return np.where(allowed0, 0.0, NEG).astype(np.float32)


def _build_program():
    nc = bacc.Bacc(
        "TRN2", target_bir_lowering=False, debug=False, num_devices=N_CORES
    )
    dt = mybir.dt.float32
    x = nc.dram_tensor("x", [M, S, S], dt, kind="ExternalInput").ap()
    mf = nc.dram_tensor("mask_first", [P, P], dt, kind="ExternalInput").ap()
    out = nc.dram_tensor("out", [M, S, S], dt, kind="ExternalOutput").ap()

    def bcast_m(ap2d, m=M):
        # (p, w) SBUF AP -> (p, m, w) with stride-0 middle dim
        (ps, pn), (ws, wn) = ap2d.ap
        return bass.AP(ap2d.tensor, ap2d.offset, [[ps, pn], [0, m], [ws, wn]])

    with tile.TileContext(nc) as tc:
        with tc.tile_pool(name="pool", bufs=1) as pool:
            # constant -1e30 background row, split memset across two engines
            c = pool.tile([P, S], dt, name="c")
            nc.vector.memset(c[:, 0 : S * 5 // 9], NEG)
            nc.gpsimd.memset(c[:, S * 5 // 9 : S], NEG)

            # block-0 mask and band data
            mf_t = pool.tile([P, P], dt, name="mf_t")
            nc.gpsimd.dma_start(mf_t[:], mf[:])
            bt0 = pool.tile([P, M, P], dt, name="bt0")
            nc.gpsimd.dma_start(
                bt0[:], x[:, 0:P, 0:P].rearrange("m p w -> p m w")
            )
            nc.vector.tensor_add(bt0[:], bt0[:], bcast_m(mf_t[:]))

            for r in range(NBLK):
                R = r * P
                # full-row constant store (elem = 8 KiB rows); alternate the
                # two HWDGE rings so transfers pipeline back-to-back
                ceng = nc.sync if r % 2 == 0 else nc.scalar
                ceng.dma_start(
                    out[:, R : R + P, :].rearrange("m p c -> p m c"),
                    bcast_m(c[:]),
                )

            # rows 0..127: computed sink+clamped-band block via SBUF (SWDGE,
            # requests mid-stream, slots between constant stores)
            nc.gpsimd.dma_start(
                out[:, 0:P, 0:P].rearrange("m p w -> p m w"), bt0[:]
            )
            # sink columns rows 128..2047: one thin DRAM->DRAM passthrough
            nc.sync.dma_start(
                out[:, P:S, 0:SINK], x[:, P:S, 0:SINK]
            )
            # band rows 128..2047: one DRAM->DRAM copy over the diagonal
            # parallelograms: out[m, r*128+p, r*128-25+p+q], q in [0, 26)
            off = P * S + (P - LEFT)
            dims = [
                [S * S, M],
                [P * (S + 1), NBLK - 1],
                [S + 1, P],
                [1, LEFT + 1],
            ]
            nc.scalar.dma_start(
                bass.AP(out.tensor, off, dims),
                bass.AP(x.tensor, off, dims),
            )

    nc.compile()
    return nc


_CACHE = {}


def _get_nc():
    if "nc" not in _CACHE:
        _CACHE["nc"] = _build_program()
    return _CACHE["nc"]


def _in_maps(w):
    mask_first = _host_masks()
    flat = w.reshape(B * H, S, S)
    return [
        {"x": flat[i * M : (i + 1) * M], "mask_first": mask_first}
        for i in range(N_CORES)
    ]


def kernel(attention_weights, seq_len=None):
    w = np.ascontiguousarray(np.asarray(attention_weights, dtype=np.float32))
    assert w.shape == (B, H, S, S)
    nc = _get_nc()
    in_maps = _in_maps(w)
    res = run_bass_kernel_spmd(nc, in_maps, core_ids=list(range(N_CORES)))
    out = np.concatenate([res.results[i]["out"] for i in range(N_CORES)], axis=0)
    return out.reshape(B, H, S, S)
